# revision 52
# baseline (speedup 1.0000x reference)
"""Trainium2 Bass kernel for the CapsuleLayer dynamic-routing module.

Strategy (8 NeuronCores, data-parallel over batch, B_local = 32/core):
  - Host pre-lays-out inputs in numpy (not part of measured HW time):
      wb  [128, (i=8, jt=9, n=10, u=16)] bf16   -- W[j,n,u,i] with j = 128*jt + p
      x2  [128, (t=18, c=128)]           bf16   -- x[b,i,j]: t=(i%2)*9+jt, c=4*b+i//2
      xn  [32, 9216]                     bf16   -- x[b, (i,j)] natural
  - u_hat is never materialized. Per routing iteration:
      s-pass:  s[b,nu] = sum_{(i,j)} (W*c)[(i,j),nu] x[b,i,j] -- 72 accumulated
               PE matmuls, k=128 j-partitions, lhsT = X2 slices, rhs = A slices.
      squash on [32, 160] tiles (exact reference semantics incl. mag over n).
      a-pass:  C[(i,j),nu] = x^T v (PE, k=b=32, 9 MMs -> one 3-bank PSUM
               tile); one ACT drain per i -> DVE z-mult (2x bf16).  u-reduce
               via fold trees on DVE: one QUAD tree for i=0..3 (amortizes
               per-op overhead) and PAIR trees for (4,5), (6,7) (keeps the
               serial tail short).  ACT (8 drains, 11.7us) and DVE (z+folds,
               ~16us) are both near-saturated here -- this is the pass's
               structural floor.
      AllReduce of the [128, 90] bf16 partial agreement (vs AllGather: +1us
               exec but the wire-back is one small DMA and the local 8-way
               fold disappears, net ~-2us on the post-collective head).
      c-pass:  cexp_u = Exp(bmat) broadcast over u in ONE ACT op (stride-0
               input AP) -- feeds the A-mults directly; A_i = Wb_i * cexp_u
               (8 DVE TTs); D[n] via PE ones-matmul on a strided u=0 view of
               cexp_u.  16 warm matmuls with rhs=cexp_u ride the PE queue
               here so the s-pass starts at the full 2.4 GHz p-state (the
               PE clock drops during the collective idle window).
  - Iteration 1 uses c == uniform (A := Wb, D := 1152) and its s-pass is
    PE-bound, so the 72 matmuls are packed 4-wide into PE column groups
    (tile_position), with a block-diagonal selector matmul summing the 4
    PSUM strips (single ACT f32->bf16 drain).
  - Input loads are spread across the sync/scalar/gpsimd DMA queues (each
    ~120 GB/s, together ~the per-core HBM port rate) with Wb_i arrival
    matched to the i-major consumption order.
  - Notes from profiling (for future work): GpSimd/Pool cannot run any
    elementwise compute in this toolchain (compiler rejects TT/STT on Pool)
    and cannot touch PSUM; SBUF collectives are disabled; the first
    collective pays ~11us of ncfw mesh setup after a host-side arming
    barrier (36-91us, jitter) that overlaps iteration-1 compute, so
    iteration-1 is largely off the critical path -- a pre-warming dummy
    collective only serializes in front of the real one and loses ~9us.
"""

import numpy as np

B, I, J, N, U = 256, 8, 1152, 10, 16
NU = N * U            # 160
ITERS = 3
NCORES = 8
BL = B // NCORES      # 32
JT = 9                # 1152 / 128
JN = JT * N           # 90

_CACHE = {}
DEBUG = False


def _build_nc():
    import concourse.bass as bass
    import concourse.bacc as bacc
    import concourse.tile as tile
    from concourse import mybir

    f32 = mybir.dt.float32
    bf16 = mybir.dt.bfloat16
    AL = mybir.AluOpType
    AF = mybir.ActivationFunctionType
    AX = mybir.AxisListType

    nc = bacc.Bacc("TRN2", target_bir_lowering=False, debug=False,
                   num_devices=NCORES)
    wb_d = nc.dram_tensor("wb", [128, I * JT * NU], bf16, kind="ExternalInput").ap()
    x2_d = nc.dram_tensor("x2", [128, 18 * 128], bf16, kind="ExternalInput").ap()
    xn_d = nc.dram_tensor("xn", [BL, I * J], bf16, kind="ExternalInput").ap()
    # block-diagonal selector: sel[p, b] = 1 iff p % 32 == b (reduces the 4
    # column-group strips of the iteration-1 s-pass via one PE matmul)
    sel_d = nc.dram_tensor("sel", [128, BL], bf16, kind="ExternalInput").ap()
    v_d = nc.dram_tensor("v", [BL, NU], f32, kind="ExternalOutput").ap()

    with tile.TileContext(nc) as tc:
        with (
            tc.tile_pool(name="big", bufs=1) as big,
            tc.tile_pool(name="abp", bufs=1) as abp,
            tc.tile_pool(name="small", bufs=2) as small,
            tc.tile_pool(name="pers", bufs=1) as pers,
            tc.tile_pool(name="ps_s", bufs=1, space="PSUM") as ps_s,
            tc.tile_pool(name="ps_c", bufs=2, space="PSUM") as ps_c,
            tc.tile_pool(name="ps_d", bufs=1, space="PSUM") as ps_d,
            tc.tile_pool(name="dram", bufs=1, space="DRAM") as dram,
        ):
            # ---------------- load inputs (spread over DMA queues) ---------
            # All three DMA queues (sync/scalar/gpsimd) run ~120 GB/s each
            # and together saturate the per-core HBM port, so the only
            # freedom is ordering: X2 gates every s-pass matmul -> split it
            # across two queues as their first transfer; Wb_i are assigned
            # so arrival order matches the i-major consumption order.
            X2 = big.tile([128, 18, 128], bf16)
            x2_v = x2_d.rearrange("p (t c) -> p t c", t=18)
            qs = [nc.sync, nc.scalar, nc.gpsimd]
            # uneven X2 stripes compensate the queue start stagger (sync
            # begins ~2.4us before gpsimd)
            for q, (lo, hi) in zip(qs, ((0, 7), (7, 13), (13, 18))):
                q.dma_start(out=X2[:, lo:hi], in_=x2_v[:, lo:hi])
            wb_v = wb_d.rearrange("p (i jt n u) -> p i jt n u",
                                  i=I, jt=JT, n=N, u=U)
            # ONE contiguous Wb tile: consecutive-i pairs are contiguous,
            # so the A-mults and z-mults can run as 4 double-size DVE ops
            # instead of 8 (halves the per-op overhead on the serial
            # A-chain).  Subtile deps let s-pass matmuls consume each
            # (i, jt) slice as its stripe lands.
            Wb_all = big.tile([128, I, JT, N, U], bf16, tag="Wall")
            for i in range(I):
                # stripe each Wb_i across all three queues so its arrival
                # tracks the cumulative-bytes floor in consumption order
                for q, (lo, hi) in zip(qs, ((0, 3), (3, 6), (6, 9))):
                    q.dma_start(out=Wb_all[:, i, lo:hi],
                                in_=wb_v[:, i, lo:hi])
            Wbs = [Wb_all[:, i] for i in range(I)]
            XN = big.tile([BL, I, J], bf16)
            nc.scalar.dma_start(out=XN, in_=xn_d.rearrange(
                "p (i j) -> p i j", i=I))

            ones = pers.tile([128, BL], bf16)
            nc.vector.memset(ones, 1.0)
            sel4 = pers.tile([128, BL], bf16, tag="sel4")
            nc.gpsimd.dma_start(out=sel4, in_=sel_d)

            # (A pairwise dummy collective to pre-absorb the one-time
            # ~11.5us first-collective arming was measured: dummy exec
            # 5.7us + 1.9us handoff, but it only wins when the arming
            # barrier ends ~19us before the iteration-1 trigger, which is
            # rare -- net negative, so no dummy.)

            # SBUF landing slots for the AllReduce wire-back
            ags0 = pers.tile([128, JN], bf16, tag="ags0")
            ags1 = pers.tile([128, JN], bf16, tag="ags1")
            ags_tiles = [ags0, ags1]

            # PE warm-up fodder: dependency-free matmuls the scheduler can
            # run while DMAs / collectives leave the PE idle, keeping the
            # HAM clock-gate at full rate for the real matmul bursts.
            warm_rhs = pers.tile([128, NU], bf16)
            nc.vector.memset(warm_rhs, 0.0)

            def warm_pe(count):
                # aliases the s-pass accumulator bank (tag "pss"); warm MMs
                # only run between the squash read and the next s-pass.
                pw = ps_s.tile([128, NU], f32, tag="pss")
                for w in range(count):
                    nc.tensor.matmul(pw[0:BL, :], lhsT=ones, rhs=warm_rhs,
                                     start=True, stop=True)

            warm_pe(16)

            X2v = X2.rearrange("p t (b ih) -> p t ih b", ih=4)
            XNv = XN.rearrange("b i (jt p) -> b i jt p", jt=JT)

            for it in range(ITERS):
                first = it == 0
                last = it == ITERS - 1

                # ------------ c-pass: A and Dinv ------------
                if first:
                    As = Wbs
                    Dinv_rep = None          # c uniform: s_sc = pss * (1/J)
                else:
                    # b after iteration t is just the running sum of the
                    # AllReduce outputs (b_0 == 0), so iteration 2 reads the
                    # wire-back tile directly and iteration 3 adds the two.
                    if it == 1:
                        bsrc = ags_tiles[0]
                    else:
                        bsum = small.tile([128, JN], f32, tag="bsum")
                        nc.vector.tensor_tensor(
                            out=bsum, in0=ags_tiles[0], in1=ags_tiles[1],
                            op=AL.add)
                        bsrc = bsum
                    # cexp_u[p, jt, n, u] = exp(b[p, jt, n]) in ONE ACT op
                    # (broadcast over u via a stride-0 input AP)
                    cexp_u = small.tile([128, JT, N, U], bf16, tag="cexpu")
                    nc.scalar.activation(
                        out=cexp_u,
                        in_=bsrc.rearrange("p (jt n) -> p jt n", jt=JT)
                        .unsqueeze(3).broadcast_to([128, JT, N, U]),
                        func=AF.Exp)
                    cexp_flat = cexp_u.rearrange("p jt n u -> p (jt n u)")
                    # A-mults as 4 double-size TTs over i-pairs (cexp
                    # replicated via a stride-0 OUTER axis; 2x mode only
                    # needs the last axis packed)
                    A_all = abp.tile([128, I, JT, N, U], bf16, tag="Aall")
                    cexp2 = cexp_flat.unsqueeze(1).broadcast_to(
                        [128, 2, JT * N * U])
                    for k in range(4):
                        nc.vector.tensor_tensor(
                            out=A_all[:, 2 * k:2 * k + 2]
                            .rearrange("p i jt n u -> p i (jt n u)"),
                            in0=Wb_all[:, 2 * k:2 * k + 2]
                            .rearrange("p i jt n u -> p i (jt n u)"),
                            in1=cexp2, op=AL.mult)
                    As = [A_all[:, i] for i in range(I)]
                    # D[n] = sum_{p,jt} cexp -> ones-matmul (replicated over
                    # the 32 batch partitions) + jt-reduce, on a strided u=0
                    # view of cexp_u.  Emitted after the A chain so engines
                    # start on A_0 first; Dinv is only consumed by the
                    # squash, well after the s-pass start.
                    psd = ps_d.tile([BL, JN], f32)
                    nc.tensor.matmul(
                        psd, lhsT=ones,
                        rhs=cexp_u[:, :, :, 0].rearrange("p jt n -> p (jt n)"),
                        start=True, stop=True)
                    D32 = small.tile([BL, N], f32, tag="d32")
                    nc.vector.tensor_reduce(
                        out=D32,
                        in_=psd.rearrange("q (jt n) -> q n jt", jt=JT),
                        axis=AX.X, op=AL.add)
                    Dinv_rep = small.tile([BL, N], f32, tag="dinv")
                    nc.vector.reciprocal(out=Dinv_rep, in_=D32)

                # ------------ s-pass: 72 accumulated matmuls ------------
                if first:
                    # Iteration 1 has no A-mult dependency, so the s-pass is
                    # PE-bound: pack 4 matmuls into the 128x128 array via
                    # column groups (M=32 each).  Each group accumulates 18
                    # of the 72 (i, jt) terms into its own 32-partition PSUM
                    # strip; the squash adds the 4 strips.
                    pss4 = ps_s.tile([128, NU], f32, tag="pss")
                    k = 0
                    for i in range(I):
                        il, ih = i % 2, i // 2
                        for jt in range(JT):
                            cg = k % 4
                            rnd = k // 4
                            nc.tensor.matmul(
                                pss4[32 * cg:32 * (cg + 1), :],
                                lhsT=X2v[:, il * JT + jt, ih, :],
                                rhs=As[i][:, jt, :, :],
                                start=(rnd == 0), stop=(rnd == 17),
                                tile_position=(0, 32 * cg))
                            k += 1
                else:
                    psst = ps_s.tile([128, NU], f32, tag="pss")
                    pss = psst[0:BL, :]
                    k = 0
                    for i in range(I):
                        il, ih = i % 2, i // 2
                        for jt in range(JT):
                            nc.tensor.matmul(
                                pss,
                                lhsT=X2v[:, il * JT + jt, ih, :],
                                rhs=As[i][:, jt, :, :],
                                start=(k == 0), stop=(k == 71))
                            k += 1

                # ------------ squash ------------
                if first:
                    # drain the 4 strips to SBUF bf16 in one ACT op, then one
                    # selector matmul sums them into [32, 160] (bank reused)
                    sb4b = small.tile([128, NU], bf16, tag="sb4b")
                    nc.scalar.copy(out=sb4b, in_=pss4)
                    pfin = ps_s.tile([128, NU], f32, tag="pss")
                    nc.tensor.matmul(pfin[0:BL, :], lhsT=sel4, rhs=sb4b,
                                     start=True, stop=True)
                    pss = pfin[0:BL, :]
                s_sc = small.tile([BL, N, U], f32, tag="ssc")
                if first:
                    nc.vector.tensor_scalar_mul(
                        out=s_sc.rearrange("b n u -> b (n u)"),
                        in0=pss, scalar1=1.0 / J)
                else:
                    nc.vector.tensor_tensor(
                        out=s_sc,
                        in0=pss.rearrange("b (n u) -> b n u", n=N),
                        in1=Dinv_rep.unsqueeze(2).broadcast_to([BL, N, U]),
                        op=AL.mult)
                sq = small.tile([BL, N, U], f32, tag="sq")
                nc.vector.tensor_tensor(out=sq, in0=s_sc, in1=s_sc,
                                        op=AL.mult)
                mag = small.tile([BL, U], f32, tag="mag")
                nc.vector.tensor_reduce(
                    out=mag, in_=sq.rearrange("b n u -> b u n"),
                    axis=AX.X, op=AL.add)
                sqrtm = small.tile([BL, U], f32, tag="sqrtm")
                nc.scalar.activation(out=sqrtm, in_=mag, func=AF.Sqrt)
                onep = small.tile([BL, U], f32, tag="onep")
                nc.vector.tensor_scalar_add(out=onep, in0=mag, scalar1=1.0)
                rec = small.tile([BL, U], f32, tag="rec")
                nc.vector.reciprocal(out=rec, in_=onep)
                g = small.tile([BL, U], f32, tag="g")
                if last:
                    nc.vector.tensor_tensor(out=g, in0=sqrtm, in1=rec,
                                            op=AL.mult)
                    v_f32 = small.tile([BL, N, U], f32, tag="vf32")
                    nc.vector.tensor_tensor(
                        out=v_f32, in0=s_sc,
                        in1=g.unsqueeze(1).broadcast_to([BL, N, U]),
                        op=AL.mult)
                    nc.sync.dma_start(
                        out=v_d, in_=v_f32.rearrange("b n u -> b (n u)"))
                    break

                # fold the 1/B mean scale into g; emit bf16 v directly
                nc.vector.scalar_tensor_tensor(
                    out=g, in0=sqrtm, scalar=1.0 / B, in1=rec,
                    op0=AL.mult, op1=AL.mult)
                vb16 = small.tile([BL, N, U], bf16, tag="vb16")
                nc.vector.tensor_tensor(
                    out=vb16, in0=s_sc,
                    in1=g.unsqueeze(1).broadcast_to([BL, N, U]),
                    op=AL.mult)
                vb16 = vb16.rearrange("b n u -> b (n u)")

                # ------------ a-pass ------------
                # Per i: 9 C-matmuls into one 3-bank PSUM tile -> one ACT
                # drain -> DVE z-mult (2x bf16).  The u-fold runs on a QUAD
                # of i (0..3) then two PAIRs (4,5), (6,7): the quad amortizes
                # per-op overhead, the pairs keep the serial tail short.
                apart = small.tile([128, JN], bf16, tag="apart")
                with nc.allow_low_precision(
                        reason="agreement wire format; partial sums held in "
                               "bf16, rounding is within tolerance"):
                    part_sums = []
                    for i in range(I):
                        psc = ps_c.tile([128, 3, 512], f32)
                        for jt in range(JT):
                            gb, kb = divmod(jt, 3)
                            nc.tensor.matmul(
                                psc[:, gb, kb * NU:(kb + 1) * NU],
                                lhsT=XNv[:, i, jt, :],
                                rhs=vb16,
                                start=True, stop=True)
                        if i == 0:
                            zt = small.tile([128, 4 * JT, N, U], bf16,
                                            tag="zquad")
                            zoff = 0
                        elif i in (4, 6):
                            zt = small.tile([128, 2 * JT, N, U], bf16,
                                            tag=f"zpair{(i - 4) // 2}")
                            zoff = i
                        if i % 2 == 0:
                            Cbp = small.tile([128, 2, 3, 3 * NU], bf16,
                                             tag="cbp")
                        nc.scalar.copy(out=Cbp[:, i % 2],
                                       in_=psc[:, :, 0:3 * NU])
                        if i % 2 == 0:
                            continue
                        # one double-size z-mult per i-pair (Wb pairs are
                        # contiguous in Wb_all; both drains land in Cbp)
                        zpair = zt[:, (i - 1 - zoff) * JT:(i + 1 - zoff) * JT]
                        nc.vector.tensor_tensor(
                            out=zpair.rearrange("p (i jt) n u -> p i (jt n u)",
                                                i=2),
                            in0=Wb_all[:, i - 1:i + 1]
                            .rearrange("p i jt n u -> p i (jt n u)"),
                            in1=Cbp.rearrange("p i g r -> p i (g r)"),
                            op=AL.mult)
                        if i not in (3, 5, 7):
                            continue
                        # u-fold tree over the current group (quad or pair)
                        gjt = 4 * JT if i == 3 else 2 * JT
                        t8 = small.tile([128, gjt, N, 8], bf16,
                                        tag=f"t8{i}")
                        nc.vector.tensor_tensor(
                            out=t8, in0=zt[:, :, :, 0:8],
                            in1=zt[:, :, :, 8:16], op=AL.add)
                        t4 = small.tile([128, gjt, N, 4], bf16, tag=f"t4{i}")
                        nc.vector.tensor_tensor(
                            out=t4, in0=t8[:, :, :, 0:4],
                            in1=t8[:, :, :, 4:8], op=AL.add)
                        t2 = small.tile([128, gjt, N, 2], bf16, tag=f"t2{i}")
                        nc.vector.tensor_tensor(
                            out=t2, in0=t4[:, :, :, 0:2],
                            in1=t4[:, :, :, 2:4], op=AL.add)
                        z1 = small.tile([128, gjt, N], bf16, tag=f"z1{i}")
                        nc.vector.tensor_tensor(
                            out=z1, in0=t2[:, :, :, 0],
                            in1=t2[:, :, :, 1], op=AL.add)
                        if i == 3:
                            # quad: fold 4*JT -> JT in two halvings
                            q2 = small.tile([128, 2 * JT, N], bf16, tag="q2")
                            nc.vector.tensor_tensor(
                                out=q2, in0=z1[:, 0:2 * JT],
                                in1=z1[:, 2 * JT:4 * JT], op=AL.add)
                            pq = small.tile([128, JN], bf16, tag="pq")
                            nc.vector.tensor_tensor(
                                out=pq,
                                in0=q2[:, 0:JT].rearrange("p a b -> p (a b)"),
                                in1=q2[:, JT:2 * JT]
                                .rearrange("p a b -> p (a b)"),
                                op=AL.add)
                            part_sums.append(pq)
                        else:
                            ps = small.tile([128, JN], bf16, tag=f"psum{i}")
                            nc.vector.tensor_tensor(
                                out=ps,
                                in0=z1[:, 0:JT].rearrange("p a b -> p (a b)"),
                                in1=z1[:, JT:2 * JT]
                                .rearrange("p a b -> p (a b)"),
                                op=AL.add)
                            part_sums.append(ps)
                        if i == 5:
                            acc = small.tile([128, JN], f32, tag="aacc")
                            nc.vector.tensor_tensor(
                                out=acc, in0=part_sums[0], in1=part_sums[1],
                                op=AL.add)
                        elif i == 7:
                            nc.vector.tensor_tensor(
                                out=apart, in0=acc, in1=part_sums[2],
                                op=AL.add)

                    # ---- ncfw AllReduce collective: one small wire-back,
                    # no local 8-way fold ----
                    ar_in = dram.tile([128, JN], bf16, tag=f"ari{it}")
                    ar_out = dram.tile([128, JN], bf16, tag=f"aro{it}")
                    nc.sync.dma_start(out=ar_in, in_=apart)
                    nc.gpsimd.collective_compute(
                        "AllReduce", AL.add,
                        ins=[ar_in.opt()], outs=[ar_out.opt()],
                        replica_groups=[list(range(NCORES))])
                    # wire-back lands in SBUF: feeds the exp directly (b is
                    # the running sum of AllReduce outputs) and the PE
                    # warm-up below.  (An SWDGE accumulate-DMA into an f32 b
                    # was tried here and measured ~1.7us slower than the
                    # HWDGE wire-back.)
                    agsum = ags_tiles[it]
                    nc.sync.dma_start(out=agsum, in_=ar_out)
                # PE clock ramp: the collective idle window drops the PE
                # p-state; these dependency-carrying warm matmuls (rhs =
                # the wire-back tile) run during the exp/A-mult window so
                # the s-pass bursts at full clock without queueing ahead
                # of the real s-pass matmuls.
                pwrm = ps_s.tile([128, NU], f32, tag="pss")
                for w in range(10):
                    nc.tensor.matmul(pwrm[0:BL, 0:JN], lhsT=ones,
                                     rhs=agsum, start=True, stop=True)

    nc.compile()
    return nc


def _prep_inputs(x_full, W):
    """Host-side relayout. x_full: [B, I, J] f32, W: [J, N, U, I] f32."""
    import ml_dtypes
    bf = ml_dtypes.bfloat16
    # Wb[p, i, jt, n, u] = W[128*jt+p, n, u, i]
    Wb = np.ascontiguousarray(
        W.reshape(JT, 128, N, U, I).transpose(1, 4, 0, 2, 3)
    ).reshape(128, I * JT * N * U).astype(bf)
    in_maps = []
    for c in range(NCORES):
        xc = x_full[c * BL:(c + 1) * BL]                   # [32, 8, 1152]
        # x128[4b+ih, il, j] = xc[b, 2*ih+il, j]
        x128 = xc.reshape(BL, 4, 2, J).reshape(128, 2, J)
        # X2[p, t=(il*9+jt), c] = x128[c, il, 128*jt+p]
        X2 = np.ascontiguousarray(
            x128.reshape(128, 2, JT, 128).transpose(3, 1, 2, 0)
        ).reshape(128, 18 * 128).astype(bf)
        xn = xc.reshape(BL, I * J).astype(bf)
        sel = np.tile(np.eye(BL, dtype=np.float32), (4, 1)).astype(bf)
        in_maps.append({"wb": Wb, "x2": X2, "xn": xn, "sel": sel})
    return in_maps


def kernel(x, W):
    """x: [256, 8, 1152] f32; W: [1152, 10, 16, 8] f32 ->
    v: [256, 10, 16, 1] f32."""
    from concourse.bass_utils import run_bass_kernel_spmd

    x = np.asarray(x, dtype=np.float32)
    W = np.asarray(W, dtype=np.float32)
    if "nc" not in _CACHE:
        _CACHE["nc"] = _build_nc()
    nc = _CACHE["nc"]
    in_maps = _prep_inputs(x, W)
    res = run_bass_kernel_spmd(nc, in_maps, core_ids=list(range(NCORES)))
    out = np.concatenate([r["v"] for r in res.results], axis=0)
    return out.reshape(B, N, U, 1).astype(np.float32)


if __name__ == "__main__":
    rng = np.random.default_rng(0)
    x = rng.standard_normal((B, I, J), dtype=np.float32)
    W = rng.standard_normal((J, N, U, I), dtype=np.float32)
    got = kernel(x, W)
    # numpy reference for a self-contained smoke test
    u_hat = np.einsum('jnui,bij->bjnu', W, x)
    b = np.zeros((J, N), dtype=np.float32)
    for _ in range(ITERS):
        e = np.exp(b - b.max(axis=0, keepdims=True))
        c = e / e.sum(axis=0, keepdims=True)
        s = np.einsum('jn,bjnu->bnu', c, u_hat)
        mag = np.sum(s * s, axis=1, keepdims=True)
        v = (mag / (1.0 + mag)) * (s / np.sqrt(mag))
        b = b + np.einsum('bjnu,bnu->jn', u_hat, v) / B
    exp = v[..., None]
    rel = np.linalg.norm(got - exp) / np.linalg.norm(exp)
    print("rel_fro:", rel)
